# revision 8
# baseline (speedup 1.0000x reference)
"""Trainium2 Bass kernel for nn_FCClassifier (predictive-coding FC network).

Data-parallel over batch (1024 -> 128 rows/core on 8 cores); state in SBUF as
[128, width] fp32. Per settling step:
  top-down:  pred_{li-1} = tanh(x_li) @ W_li^T   (bf16 operands, fp32 PSUM acc)
             e_{li-1} = x_{li-1} - pred + noise_eff
  bottom-up: g_li = e_{li-1} @ W_li ; x_li += 0.1*(g*(1-tanh(x_li)^2) - e_li)
noise_eff is host-precomputed (exact jax threefry bits, 0.034*temp scaling and
bias b_li folded in). Weights stream from HBM every step as two pre-swizzled
bf16 blobs (W^T layout for top-down, natural for bottom-up/init). Transposed
activations (matmul stationary operands) are made on-chip with PE transposes.
Steps run in a hardware For_i loop; only the noise DMA offset is step-dependent.
"""
import contextlib
import numpy as np
import ml_dtypes

import concourse.bass as bass
import concourse.tile as tile
from concourse import bacc, mybir
from concourse import bass_utils

SIZES = [3072, 4096, 4096, 2048, 1000]
BATCH = 1024
GAMMA = 0.1
NOISE_SCALE = 0.034
N_CORES = 8
ROWS = BATCH // N_CORES  # 128

BF16 = mybir.dt.bfloat16
F32 = mybir.dt.float32
NPBF16 = ml_dtypes.bfloat16

NGROUP = 4   # accumulator banks per matmul group
CHUNK = 512  # moving free dim / vector chunk
KSEG = 3     # k-tiles per weight-stream DMA segment

W_TOT = sum(SIZES)        # 14312
E_TOT = sum(SIZES[:4])    # 13312
XOFS = np.cumsum([0] + SIZES)
EOFS = np.cumsum([0] + SIZES[:4])


def _cdiv(a, b):
    return (a + b - 1) // b


def _chunks(total, size):
    return [(o, min(size, total - o)) for o in range(0, total, size)]


def _groups(lst, n):
    return [lst[i:i + n] for i in range(0, len(lst), n)]


def _sweep_dims(kind, li):
    if kind == "fwd":
        return SIZES[li], SIZES[li - 1]   # K, N
    return SIZES[li - 1], SIZES[li]


N_BLOCKS = sum(_cdiv(_sweep_dims("fwd", li)[0], 128) * len(_chunks(_sweep_dims("fwd", li)[1], CHUNK))
               for li in (1, 2, 3, 4))  # same count for both sweeps (608)


# ---------------------------------------------------------------- host prep

def _noise_eff(steps, bs):
    """[steps*1024, 13312] bf16: exact reference noise * scale - bias folds."""
    import jax, jax.numpy as jnp
    cpu = jax.devices("cpu")[0]
    with jax.default_device(cpu):
        nkey = jax.random.key(42)
        rows = []
        for i in range(steps):
            temp = np.float32(1.0 - np.float32(i) / steps)
            pieces = []
            for lo in range(4):
                k = jax.random.fold_in(jax.random.fold_in(nkey, i), lo)
                nz = np.asarray(jax.random.normal(k, (BATCH, SIZES[lo]), jnp.float32))
                nz = nz * np.float32(NOISE_SCALE) * temp - bs[lo][None, :]
                pieces.append(nz)
            rows.append(np.concatenate(pieces, axis=1))
        return np.stack(rows).astype(NPBF16)  # [steps, 1024, E_TOT]


def _pack_blob(Ws, kind):
    """Pre-swizzled weight blob in exact consumption order."""
    blocks = []
    order = (4, 3, 2, 1) if kind == "fwd" else (1, 2, 3, 4)
    for li in order:
        Wm = Ws[li - 1].T if kind == "fwd" else Ws[li - 1]
        K, N = Wm.shape
        kt_n = _cdiv(K, 128)
        for grp in _groups(_chunks(N, CHUNK), NGROUP):
            for seg0 in range(0, kt_n, KSEG):
                for kt in range(seg0, min(seg0 + KSEG, kt_n)):
                    k0, kw = kt * 128, min(128, K - kt * 128)
                    for (n0, nw) in grp:
                        blk = np.zeros((128, CHUNK), np.float32)
                        blk[:kw, :nw] = Wm[k0:k0 + kw, n0:n0 + nw]
                        blocks.append(blk.reshape(1, -1))
    return np.concatenate(blocks, 0).astype(NPBF16)


# ---------------------------------------------------------------- builder

class _C:
    pass


def _emit_layer_mm(c, li, kind, lhsT, blob, ofs, out_cb):
    """Matmuls for one layer of a sweep, streaming weights in KSEG segments."""
    nc = c.nc
    K, N = _sweep_dims(kind, li)
    kt_n = _cdiv(K, 128)
    for grp in _groups(_chunks(N, CHUNK), NGROUP):
        accs = [c.apool.tile([128, CHUNK], F32, tag="acc", name="acc") for _ in grp]
        for seg0 in range(0, kt_n, KSEG):
            seg_n = min(KSEG, kt_n - seg0)
            nblk = seg_n * len(grp)
            wt = c.wpool.tile([128, KSEG * NGROUP * CHUNK], BF16, tag="wstream")
            nc.sync.dma_start(
                wt[:].rearrange("p (b f) -> p b f", f=CHUNK)[:, :nblk],
                blob[ofs[0]:ofs[0] + nblk].rearrange("b (p f) -> p b f", p=128))
            ofs[0] += nblk
            for si in range(seg_n):
                kt = seg0 + si
                kw = min(128, K - kt * 128)
                for gi, (n0, nw) in enumerate(grp):
                    bi = si * len(grp) + gi
                    nc.tensor.matmul(
                        accs[gi][:, :nw],
                        lhsT[:kw, kt * 128:kt * 128 + 128],
                        wt[:kw, bi * CHUNK:bi * CHUNK + nw],
                        start=(kt == 0), stop=(kt == kt_n - 1))
        for gi, (n0, nw) in enumerate(grp):
            out_cb(li, n0, nw, accs[gi])


def _emit_transpose(c, dst, src, width):
    """PE-transpose [128, width] bf16 src -> dst [128, kt_n*128] k-major tiles."""
    nc = c.nc
    kt_n = _cdiv(width, 128)
    for base in range(0, kt_n, 8):
        nt = min(8, kt_n - base)
        pt = c.tpool.tile([128, 8 * 128], BF16, tag="tr")
        kws = []
        for j in range(nt):
            kt = base + j
            kw = min(128, width - kt * 128)
            kws.append(kw)
            nc.tensor.transpose(
                pt[:kw, j * 128:j * 128 + 128],
                src[:, kt * 128:kt * 128 + kw],
                c.ident[:, :])
        if all(k == 128 for k in kws):
            nc.vector.tensor_copy(dst[:, base * 128:(base + nt) * 128],
                                  pt[:, :nt * 128])
        else:
            nfull = sum(1 for k in kws if k == 128)
            if nfull:
                nc.vector.tensor_copy(dst[:, base * 128:(base + nfull) * 128],
                                      pt[:, :nfull * 128])
            for j in range(nfull, nt):
                kw = kws[j]
                nc.vector.tensor_copy(
                    dst[:kw, (base + j) * 128:(base + j) * 128 + 128],
                    pt[:kw, j * 128:j * 128 + 128])


def build(steps):
    nc = bacc.Bacc("TRN2", target_bir_lowering=False, debug=False,
                   num_devices=N_CORES)
    c = _C()
    c.nc = nc

    obs_d = nc.dram_tensor("obs", [ROWS, SIZES[0]], F32, kind="ExternalInput").ap()
    obsT_d = nc.dram_tensor("obsT", [SIZES[0] // 128, 128 * ROWS], BF16,
                            kind="ExternalInput").ap()
    wt_d = nc.dram_tensor("wt_blob", [N_BLOCKS, 128 * CHUNK], BF16,
                          kind="ExternalInput").ap()
    wn_d = nc.dram_tensor("wn_blob", [N_BLOCKS, 128 * CHUNK], BF16,
                          kind="ExternalInput").ap()
    ident_d = nc.dram_tensor("ident", [128, 128], BF16, kind="ExternalInput").ap()
    noise_d = nc.dram_tensor("noise", [steps * ROWS, E_TOT], BF16,
                             kind="ExternalInput").ap()
    out_d = nc.dram_tensor("out", [ROWS, SIZES[4]], F32, kind="ExternalOutput").ap()

    with tile.TileContext(nc) as tc, contextlib.ExitStack() as st:
        c.wpool = st.enter_context(tc.tile_pool(name="wstream", bufs=2))
        c.apool = st.enter_context(tc.tile_pool(name="acc", bufs=6, space="PSUM"))
        c.tpool = st.enter_context(tc.tile_pool(name="tr", bufs=2, space="PSUM"))
        sp = st.enter_context(tc.tile_pool(name="state", bufs=1))
        sc = st.enter_context(tc.tile_pool(name="scratch", bufs=2))
        tp = st.enter_context(tc.tile_pool(name="tTpool", bufs=2))

        x = sp.tile([128, W_TOT], F32)
        e = sp.tile([128, E_TOT], BF16)
        ident = sp.tile([128, 128], BF16)
        c.ident = ident
        nc.sync.dma_start(ident[:], ident_d)
        eT = {lo: sp.tile([128, _cdiv(SIZES[lo], 128) * 128], BF16, tag=f"eT{lo}", name=f"eT{lo}")
              for lo in range(4)}

        def x_ap(li, n0=0, nw=None):
            nw = SIZES[li] if nw is None else nw
            o = int(XOFS[li]) + n0
            return x[:, o:o + nw]

        def e_ap(lo, n0=0, nw=None):
            nw = SIZES[lo] if nw is None else nw
            o = int(EOFS[lo]) + n0
            return e[:, o:o + nw]

        nc.sync.dma_start(x_ap(0), obs_d)

        # ---------------- init: x_li = x_{li-1} @ W_li
        def init_out(li, n0, nw, acc):
            nc.vector.tensor_copy(x_ap(li, n0, nw), acc[:, :nw])

        init_ofs = [0]
        xT_prev = sp.tile([128, 32 * 128], BF16, tag="xTinit", name="xTinit")
        nc.sync.dma_start(
            xT_prev[:, :SIZES[0]].rearrange("p (k f) -> p k f", f=128),
            obsT_d.rearrange("k (p f) -> p k f", p=128))
        for li in (1, 2, 3, 4):
            _emit_layer_mm(c, li, "init", xT_prev, wn_d, init_ofs, init_out)
            if li < 4:
                xT_prev = sp.tile([128, 32 * 128], BF16, tag="xTinit", name="xTinit")
                for (s0, swd) in _chunks(SIZES[li], 1024):
                    xb = sc.tile([128, 1024], BF16, tag="xbinit", name="xbinit")
                    nc.vector.tensor_copy(xb[:, :swd], x_ap(li, s0, swd))
                    _emit_transpose(c, xT_prev[:, s0:s0 + _cdiv(swd, 128) * 128], xb[:, :swd], swd)

        # ---------------- settling steps
        def step_body(i):
            fwd_ofs = [0]
            for li in (4, 3, 2, 1):
                tT = tp.tile([128, 32 * 128], BF16, tag="tT", name="tT")
                for (s0, swd) in _chunks(SIZES[li], 1024):
                    tb = sc.tile([128, 1024], BF16, tag="tcast", name="tcast")
                    for (n0, nw) in _chunks(swd, CHUNK):
                        nc.scalar.activation(tb[:, n0:n0 + nw],
                                             x_ap(li, s0 + n0, nw),
                                             mybir.ActivationFunctionType.Tanh)
                    _emit_transpose(c, tT[:, s0:s0 + _cdiv(swd, 128) * 128], tb[:, :swd], swd)

                def fwd_out(li_, n0, nw, acc, _lo=li - 1):
                    nz = sc.tile([128, CHUNK], BF16, tag="nz")
                    nc.sync.dma_start(
                        nz[:, :nw],
                        noise_d[bass.ts(i, ROWS),
                                int(EOFS[_lo]) + n0:int(EOFS[_lo]) + n0 + nw])
                    # e = (pred * -1 + x) + noise_eff
                    nc.vector.scalar_tensor_tensor(
                        e_ap(_lo, n0, nw), acc[:, :nw], -1.0, x_ap(_lo, n0, nw),
                        mybir.AluOpType.mult, mybir.AluOpType.add)
                    nc.vector.tensor_add(e_ap(_lo, n0, nw), e_ap(_lo, n0, nw),
                                         nz[:, :nw])

                _emit_layer_mm(c, li, "fwd", tT, wt_d, fwd_ofs, fwd_out)
                _emit_transpose(c, eT[li - 1], e_ap(li - 1), SIZES[li - 1])

            bwd_ofs = [0]

            def bwd_out(li, n0, nw, acc):
                t2 = sc.tile([128, CHUNK], F32, tag="t2")
                nc.scalar.activation(t2[:, :nw], x_ap(li, n0, nw),
                                     mybir.ActivationFunctionType.Tanh)
                nc.scalar.activation(t2[:, :nw], t2[:, :nw],
                                     mybir.ActivationFunctionType.Square)
                nc.vector.tensor_scalar(t2[:, :nw], t2[:, :nw], -1.0, 1.0,
                                        mybir.AluOpType.mult, mybir.AluOpType.add)
                gd = sc.tile([128, CHUNK], F32, tag="gd")
                nc.vector.tensor_mul(gd[:, :nw], acc[:, :nw], t2[:, :nw])
                if li < 4:  # e4 is identically zero in the reference
                    nc.vector.scalar_tensor_tensor(
                        gd[:, :nw], e_ap(li, n0, nw), -1.0, gd[:, :nw],
                        mybir.AluOpType.mult, mybir.AluOpType.add)
                nc.vector.scalar_tensor_tensor(
                    x_ap(li, n0, nw), gd[:, :nw], GAMMA, x_ap(li, n0, nw),
                    mybir.AluOpType.mult, mybir.AluOpType.add)

            for li in (1, 2, 3, 4):
                _emit_layer_mm(c, li, "bwd", eT[li - 1], wn_d, bwd_ofs, bwd_out)

        with tc.For_i(0, steps, 1) as i:
            step_body(i)

        nc.sync.dma_start(out_d, x_ap(4))
    nc.finalize()
    return nc


# ---------------------------------------------------------------- entry

_CACHE = {}


def kernel(**inputs):
    obs = np.asarray(inputs["obs"], np.float32)
    Ws = [np.asarray(inputs[f"W{i}"], np.float32) for i in range(1, 5)]
    bs = [np.asarray(inputs[f"b{i}"], np.float32) for i in range(1, 5)]
    steps = int(inputs["steps"])
    assert obs.shape == (BATCH, SIZES[0])

    if steps not in _CACHE:
        _CACHE[steps] = build(steps)
    nc = _CACHE[steps]

    noise = _noise_eff(steps, bs)  # [steps, 1024, E_TOT] bf16
    wt_blob = _pack_blob(Ws, "fwd")
    wn_blob = _pack_blob(Ws, "bwd")
    ident = np.eye(128, dtype=NPBF16)

    in_maps = []
    for cx in range(N_CORES):
        r0 = cx * ROWS
        obs_c = np.ascontiguousarray(obs[r0:r0 + ROWS])
        obsT_c = np.ascontiguousarray(
            obs_c.T.astype(NPBF16).reshape(SIZES[0] // 128, 128 * ROWS))
        nz_c = np.ascontiguousarray(
            noise[:, r0:r0 + ROWS, :]).reshape(steps * ROWS, E_TOT)
        in_maps.append({
            "obs": obs_c, "obsT": obsT_c, "wt_blob": wt_blob,
            "wn_blob": wn_blob, "ident": ident, "noise": nz_c,
        })

    res = bass_utils.run_bass_kernel_spmd(
        nc, in_maps, core_ids=list(range(N_CORES)), trace=False)
    return np.concatenate(
        [res.results[cx]["out"] for cx in range(N_CORES)], 0).astype(np.float32)


# revision 9
# speedup vs baseline: 1.2743x; 1.2743x over previous
"""Trainium2 Bass kernel for nn_FCClassifier (predictive-coding FC network).

Data-parallel over batch (1024 -> 128 rows/core on 8 cores); state in SBUF as
[128, width] fp32. Per settling step:
  top-down:  pred_{li-1} = tanh(x_li) @ W_li^T   (bf16 operands, fp32 PSUM acc)
             e_{li-1} = x_{li-1} - pred + noise_eff
  bottom-up: g_li = e_{li-1} @ W_li ; x_li += 0.1*(g*(1-tanh(x_li)^2) - e_li)
noise_eff is host-precomputed (exact jax threefry bits, 0.034*temp scaling and
bias b_li folded in). Weights stream from HBM every step as two pre-swizzled
bf16 blobs (W^T layout for top-down, natural for bottom-up/init). Transposed
activations (matmul stationary operands) are made on-chip with PE transposes.
Steps run in a hardware For_i loop; only the noise DMA offset is step-dependent.
"""
import contextlib
import numpy as np
import ml_dtypes

import concourse.bass as bass
import concourse.tile as tile
from concourse import bacc, mybir
from concourse import bass_utils

SIZES = [3072, 4096, 4096, 2048, 1000]
BATCH = 1024
GAMMA = 0.1
NOISE_SCALE = 0.034
N_CORES = 8
ROWS = BATCH // N_CORES  # 128

BF16 = mybir.dt.bfloat16
F32 = mybir.dt.float32
NPBF16 = ml_dtypes.bfloat16

NGROUP = 4   # accumulator banks per matmul group
CHUNK = 512  # moving free dim / vector chunk
KSEG = 3     # k-tiles per weight-stream DMA segment

W_TOT = sum(SIZES)        # 14312
E_TOT = sum(SIZES[:4])    # 13312
XOFS = np.cumsum([0] + SIZES)
EOFS = np.cumsum([0] + SIZES[:4])


def _cdiv(a, b):
    return (a + b - 1) // b


def _chunks(total, size):
    return [(o, min(size, total - o)) for o in range(0, total, size)]


def _groups(lst, n):
    return [lst[i:i + n] for i in range(0, len(lst), n)]


def _sweep_dims(kind, li):
    if kind == "fwd":
        return SIZES[li], SIZES[li - 1]   # K, N
    return SIZES[li - 1], SIZES[li]


N_BLOCKS = sum(_cdiv(_sweep_dims("fwd", li)[0], 128) * len(_chunks(_sweep_dims("fwd", li)[1], CHUNK))
               for li in (1, 2, 3, 4))  # same count for both sweeps (608)


# ---------------------------------------------------------------- host prep

def _noise_eff(steps, bs):
    """[steps*1024, 13312] bf16: exact reference noise * scale - bias folds."""
    import jax, jax.numpy as jnp
    cpu = jax.devices("cpu")[0]
    with jax.default_device(cpu):
        nkey = jax.random.key(42)
        rows = []
        for i in range(steps):
            temp = np.float32(1.0 - np.float32(i) / steps)
            pieces = []
            for lo in range(4):
                k = jax.random.fold_in(jax.random.fold_in(nkey, i), lo)
                nz = np.asarray(jax.random.normal(k, (BATCH, SIZES[lo]), jnp.float32))
                nz = nz * np.float32(NOISE_SCALE) * temp - bs[lo][None, :]
                pieces.append(nz)
            rows.append(np.concatenate(pieces, axis=1))
        return np.stack(rows).astype(NPBF16)  # [steps, 1024, E_TOT]


def _pack_blob(Ws, kind):
    """Pre-swizzled weight blob in exact consumption order."""
    blocks = []
    order = (4, 3, 2, 1) if kind == "fwd" else (1, 2, 3, 4)
    for li in order:
        Wm = Ws[li - 1].T if kind == "fwd" else Ws[li - 1]
        K, N = Wm.shape
        kt_n = _cdiv(K, 128)
        for grp in _groups(_chunks(N, CHUNK), NGROUP):
            for seg0 in range(0, kt_n, KSEG):
                for kt in range(seg0, min(seg0 + KSEG, kt_n)):
                    k0, kw = kt * 128, min(128, K - kt * 128)
                    for (n0, nw) in grp:
                        blk = np.zeros((128, CHUNK), np.float32)
                        blk[:kw, :nw] = Wm[k0:k0 + kw, n0:n0 + nw]
                        blocks.append(blk.reshape(1, -1))
    return np.concatenate(blocks, 0).astype(NPBF16)


# ---------------------------------------------------------------- builder

class _C:
    pass


def _emit_layer_mm(c, li, kind, lhsT, blob, ofs, out_cb):
    """Matmuls for one layer of a sweep, streaming weights in KSEG segments."""
    nc = c.nc
    K, N = _sweep_dims(kind, li)
    kt_n = _cdiv(K, 128)
    for grp in _groups(_chunks(N, CHUNK), NGROUP):
        accs = [c.apool.tile([128, CHUNK], F32, tag="acc", name="acc") for _ in grp]
        for seg0 in range(0, kt_n, KSEG):
            seg_n = min(KSEG, kt_n - seg0)
            nblk = seg_n * len(grp)
            wt = c.wpool.tile([128, KSEG * NGROUP * CHUNK], BF16, tag="wstream")
            nc.sync.dma_start(
                wt[:].rearrange("p (b f) -> p b f", f=CHUNK)[:, :nblk],
                blob[ofs[0]:ofs[0] + nblk].rearrange("b (p f) -> p b f", p=128))
            ofs[0] += nblk
            for si in range(seg_n):
                kt = seg0 + si
                kw = min(128, K - kt * 128)
                for gi, (n0, nw) in enumerate(grp):
                    bi = si * len(grp) + gi
                    nc.tensor.matmul(
                        accs[gi][:, :nw],
                        lhsT[:kw, kt * 128:kt * 128 + 128],
                        wt[:kw, bi * CHUNK:bi * CHUNK + nw],
                        start=(kt == 0), stop=(kt == kt_n - 1))
        for gi, (n0, nw) in enumerate(grp):
            out_cb(li, n0, nw, accs[gi])


def _emit_transpose(c, dst, src, width):
    """PE-transpose [128, width] bf16 src -> dst [128, kt_n*128] k-major tiles."""
    nc = c.nc
    kt_n = _cdiv(width, 128)
    for base in range(0, kt_n, 8):
        nt = min(8, kt_n - base)
        pt = c.tpool.tile([128, 8 * 128], BF16, tag="tr")
        kws = []
        for j in range(nt):
            kt = base + j
            kw = min(128, width - kt * 128)
            kws.append(kw)
            nc.tensor.transpose(
                pt[:kw, j * 128:j * 128 + 128],
                src[:, kt * 128:kt * 128 + kw],
                c.ident[:, :])
        if all(k == 128 for k in kws):
            nc.vector.tensor_copy(dst[:, base * 128:(base + nt) * 128],
                                  pt[:, :nt * 128])
        else:
            nfull = sum(1 for k in kws if k == 128)
            if nfull:
                nc.vector.tensor_copy(dst[:, base * 128:(base + nfull) * 128],
                                      pt[:, :nfull * 128])
            for j in range(nfull, nt):
                kw = kws[j]
                nc.vector.tensor_copy(
                    dst[:kw, (base + j) * 128:(base + j) * 128 + 128],
                    pt[:kw, j * 128:j * 128 + 128])


def build(steps):
    nc = bacc.Bacc("TRN2", target_bir_lowering=False, debug=False,
                   num_devices=N_CORES)
    c = _C()
    c.nc = nc

    obs_d = nc.dram_tensor("obs", [ROWS, SIZES[0]], F32, kind="ExternalInput").ap()
    obsT_d = nc.dram_tensor("obsT", [SIZES[0] // 128, 128 * ROWS], BF16,
                            kind="ExternalInput").ap()
    wt_d = nc.dram_tensor("wt_blob", [N_BLOCKS, 128 * CHUNK], BF16,
                          kind="ExternalInput").ap()
    wn_d = nc.dram_tensor("wn_blob", [N_BLOCKS, 128 * CHUNK], BF16,
                          kind="ExternalInput").ap()
    ident_d = nc.dram_tensor("ident", [128, 128], BF16, kind="ExternalInput").ap()
    noise_d = nc.dram_tensor("noise", [steps * ROWS, E_TOT], BF16,
                             kind="ExternalInput").ap()
    out_d = nc.dram_tensor("out", [ROWS, SIZES[4]], F32, kind="ExternalOutput").ap()

    with tile.TileContext(nc) as tc, contextlib.ExitStack() as st:
        c.wpool = st.enter_context(tc.tile_pool(name="wstream", bufs=3))
        c.apool = st.enter_context(tc.tile_pool(name="acc", bufs=6, space="PSUM"))
        c.tpool = st.enter_context(tc.tile_pool(name="tr", bufs=2, space="PSUM"))
        sp = st.enter_context(tc.tile_pool(name="state", bufs=1))
        sc = st.enter_context(tc.tile_pool(name="scratch", bufs=2))
        tp = st.enter_context(tc.tile_pool(name="tTpool", bufs=2))

        x = sp.tile([128, W_TOT], F32)
        e = sp.tile([128, E_TOT], BF16)
        ident = sp.tile([128, 128], BF16)
        c.ident = ident
        nc.sync.dma_start(ident[:], ident_d)
        eT = {lo: sp.tile([128, _cdiv(SIZES[lo], 128) * 128], BF16, tag=f"eT{lo}", name=f"eT{lo}")
              for lo in range(4)}

        def x_ap(li, n0=0, nw=None):
            nw = SIZES[li] if nw is None else nw
            o = int(XOFS[li]) + n0
            return x[:, o:o + nw]

        def e_ap(lo, n0=0, nw=None):
            nw = SIZES[lo] if nw is None else nw
            o = int(EOFS[lo]) + n0
            return e[:, o:o + nw]

        nc.sync.dma_start(x_ap(0), obs_d)

        # ---------------- init: x_li = x_{li-1} @ W_li
        def init_out(li, n0, nw, acc):
            nc.vector.tensor_copy(x_ap(li, n0, nw), acc[:, :nw])

        init_ofs = [0]
        xT_prev = sp.tile([128, 32 * 128], BF16, tag="xTinit", name="xTinit")
        nc.sync.dma_start(
            xT_prev[:, :SIZES[0]].rearrange("p (k f) -> p k f", f=128),
            obsT_d.rearrange("k (p f) -> p k f", p=128))
        for li in (1, 2, 3, 4):
            _emit_layer_mm(c, li, "init", xT_prev, wn_d, init_ofs, init_out)
            if li < 4:
                xT_prev = sp.tile([128, 32 * 128], BF16, tag="xTinit", name="xTinit")
                for (s0, swd) in _chunks(SIZES[li], 1024):
                    xb = sc.tile([128, 1024], BF16, tag="xbinit", name="xbinit")
                    nc.vector.tensor_copy(xb[:, :swd], x_ap(li, s0, swd))
                    _emit_transpose(c, xT_prev[:, s0:s0 + _cdiv(swd, 128) * 128], xb[:, :swd], swd)

        # ---------------- settling steps
        def step_body(i):
            fwd_ofs = [0]
            for li in (4, 3, 2, 1):
                tT = tp.tile([128, 32 * 128], BF16, tag="tT", name="tT")
                for (s0, swd) in _chunks(SIZES[li], 1024):
                    tb = sc.tile([128, 1024], BF16, tag="tcast", name="tcast")
                    for (n0, nw) in _chunks(swd, CHUNK):
                        nc.scalar.activation(tb[:, n0:n0 + nw],
                                             x_ap(li, s0 + n0, nw),
                                             mybir.ActivationFunctionType.Tanh)
                    _emit_transpose(c, tT[:, s0:s0 + _cdiv(swd, 128) * 128], tb[:, :swd], swd)

                def fwd_out(li_, n0, nw, acc, _lo=li - 1):
                    nz = sc.tile([128, CHUNK], BF16, tag="nz")
                    nc.sync.dma_start(
                        nz[:, :nw],
                        noise_d[bass.ts(i, ROWS),
                                int(EOFS[_lo]) + n0:int(EOFS[_lo]) + n0 + nw])
                    # e = (pred * -1 + x) + noise_eff
                    nc.vector.scalar_tensor_tensor(
                        e_ap(_lo, n0, nw), acc[:, :nw], -1.0, x_ap(_lo, n0, nw),
                        mybir.AluOpType.mult, mybir.AluOpType.add)
                    nc.vector.tensor_add(e_ap(_lo, n0, nw), e_ap(_lo, n0, nw),
                                         nz[:, :nw])

                _emit_layer_mm(c, li, "fwd", tT, wt_d, fwd_ofs, fwd_out)
                _emit_transpose(c, eT[li - 1], e_ap(li - 1), SIZES[li - 1])

            bwd_ofs = [0]

            def bwd_out(li, n0, nw, acc):
                t2 = sc.tile([128, CHUNK], F32, tag="t2")
                nc.scalar.activation(t2[:, :nw], x_ap(li, n0, nw),
                                     mybir.ActivationFunctionType.Tanh)
                nc.scalar.activation(t2[:, :nw], t2[:, :nw],
                                     mybir.ActivationFunctionType.Square)
                nc.vector.tensor_scalar(t2[:, :nw], t2[:, :nw], -1.0, 1.0,
                                        mybir.AluOpType.mult, mybir.AluOpType.add)
                gd = sc.tile([128, CHUNK], F32, tag="gd")
                nc.vector.tensor_mul(gd[:, :nw], acc[:, :nw], t2[:, :nw])
                if li < 4:  # e4 is identically zero in the reference
                    nc.vector.scalar_tensor_tensor(
                        gd[:, :nw], e_ap(li, n0, nw), -1.0, gd[:, :nw],
                        mybir.AluOpType.mult, mybir.AluOpType.add)
                nc.vector.scalar_tensor_tensor(
                    x_ap(li, n0, nw), gd[:, :nw], GAMMA, x_ap(li, n0, nw),
                    mybir.AluOpType.mult, mybir.AluOpType.add)

            for li in (1, 2, 3, 4):
                _emit_layer_mm(c, li, "bwd", eT[li - 1], wn_d, bwd_ofs, bwd_out)

        with tc.For_i(0, steps, 1, hint_engines=(mybir.EngineType.PE, mybir.EngineType.DVE, mybir.EngineType.Activation, mybir.EngineType.SP)) as i:
            step_body(i)

        nc.sync.dma_start(out_d, x_ap(4))
    nc.finalize()
    return nc


# ---------------------------------------------------------------- entry

_CACHE = {}


def kernel(**inputs):
    obs = np.asarray(inputs["obs"], np.float32)
    Ws = [np.asarray(inputs[f"W{i}"], np.float32) for i in range(1, 5)]
    bs = [np.asarray(inputs[f"b{i}"], np.float32) for i in range(1, 5)]
    steps = int(inputs["steps"])
    assert obs.shape == (BATCH, SIZES[0])

    if steps not in _CACHE:
        _CACHE[steps] = build(steps)
    nc = _CACHE[steps]

    noise = _noise_eff(steps, bs)  # [steps, 1024, E_TOT] bf16
    wt_blob = _pack_blob(Ws, "fwd")
    wn_blob = _pack_blob(Ws, "bwd")
    ident = np.eye(128, dtype=NPBF16)

    in_maps = []
    for cx in range(N_CORES):
        r0 = cx * ROWS
        obs_c = np.ascontiguousarray(obs[r0:r0 + ROWS])
        obsT_c = np.ascontiguousarray(
            obs_c.T.astype(NPBF16).reshape(SIZES[0] // 128, 128 * ROWS))
        nz_c = np.ascontiguousarray(
            noise[:, r0:r0 + ROWS, :]).reshape(steps * ROWS, E_TOT)
        in_maps.append({
            "obs": obs_c, "obsT": obsT_c, "wt_blob": wt_blob,
            "wn_blob": wn_blob, "ident": ident, "noise": nz_c,
        })

    res = bass_utils.run_bass_kernel_spmd(
        nc, in_maps, core_ids=list(range(N_CORES)), trace=False)
    return np.concatenate(
        [res.results[cx]["out"] for cx in range(N_CORES)], 0).astype(np.float32)


# revision 10
# speedup vs baseline: 1.3069x; 1.0256x over previous
"""Trainium2 Bass kernel for nn_FCClassifier (predictive-coding FC network).

Data-parallel over batch (1024 -> 128 rows/core on 8 cores); state in SBUF as
[128, width] fp32. Per settling step:
  top-down:  pred_{li-1} = tanh(x_li) @ W_li^T   (bf16 operands, fp32 PSUM acc)
             e_{li-1} = x_{li-1} - pred + noise_eff
  bottom-up: g_li = e_{li-1} @ W_li ; x_li += 0.1*(g*(1-tanh(x_li)^2) - e_li)
noise_eff is host-precomputed (exact jax threefry bits, 0.034*temp scaling and
bias b_li folded in). Weights stream from HBM every step as two pre-swizzled
bf16 blobs (W^T layout for top-down, natural for bottom-up/init). Transposed
activations (matmul stationary operands) are made on-chip with PE transposes.
Steps run in a hardware For_i loop; only the noise DMA offset is step-dependent.
"""
import contextlib
import numpy as np
import ml_dtypes

import concourse.bass as bass
import concourse.tile as tile
from concourse import bacc, mybir
from concourse import bass_utils

SIZES = [3072, 4096, 4096, 2048, 1000]
BATCH = 1024
GAMMA = 0.1
NOISE_SCALE = 0.034
N_CORES = 8
ROWS = BATCH // N_CORES  # 128

BF16 = mybir.dt.bfloat16
F32 = mybir.dt.float32
NPBF16 = ml_dtypes.bfloat16

NGROUP = 4   # accumulator banks per matmul group
CHUNK = 512  # moving free dim / vector chunk
KSEG = 3     # k-tiles per weight-stream DMA segment

W_TOT = sum(SIZES)        # 14312
E_TOT = sum(SIZES[:4])    # 13312
XOFS = np.cumsum([0] + SIZES)
EOFS = np.cumsum([0] + SIZES[:4])


def _cdiv(a, b):
    return (a + b - 1) // b


def _chunks(total, size):
    return [(o, min(size, total - o)) for o in range(0, total, size)]


def _groups(lst, n):
    return [lst[i:i + n] for i in range(0, len(lst), n)]


def _sweep_dims(kind, li):
    if kind == "fwd":
        return SIZES[li], SIZES[li - 1]   # K, N
    return SIZES[li - 1], SIZES[li]


N_BLOCKS = sum(_cdiv(_sweep_dims("fwd", li)[0], 128) * len(_chunks(_sweep_dims("fwd", li)[1], CHUNK))
               for li in (1, 2, 3, 4))  # same count for both sweeps (608)


# ---------------------------------------------------------------- host prep

def _noise_eff(steps, bs):
    """[steps*1024, 13312] bf16: exact reference noise * scale - bias folds."""
    import jax, jax.numpy as jnp
    cpu = jax.devices("cpu")[0]
    with jax.default_device(cpu):
        nkey = jax.random.key(42)
        rows = []
        for i in range(steps):
            temp = np.float32(1.0 - np.float32(i) / steps)
            pieces = []
            for lo in range(4):
                k = jax.random.fold_in(jax.random.fold_in(nkey, i), lo)
                nz = np.asarray(jax.random.normal(k, (BATCH, SIZES[lo]), jnp.float32))
                nz = nz * np.float32(NOISE_SCALE) * temp - bs[lo][None, :]
                pieces.append(nz)
            rows.append(np.concatenate(pieces, axis=1))
        return np.stack(rows).astype(NPBF16)  # [steps, 1024, E_TOT]


def _pack_blob(Ws, kind):
    """Pre-swizzled weight blob in exact consumption order."""
    blocks = []
    order = (4, 3, 2, 1) if kind == "fwd" else (1, 2, 3, 4)
    for li in order:
        Wm = Ws[li - 1].T if kind == "fwd" else Ws[li - 1]
        K, N = Wm.shape
        kt_n = _cdiv(K, 128)
        for grp in _groups(_chunks(N, CHUNK), NGROUP):
            for seg0 in range(0, kt_n, KSEG):
                for kt in range(seg0, min(seg0 + KSEG, kt_n)):
                    k0, kw = kt * 128, min(128, K - kt * 128)
                    for (n0, nw) in grp:
                        blk = np.zeros((128, CHUNK), np.float32)
                        blk[:kw, :nw] = Wm[k0:k0 + kw, n0:n0 + nw]
                        blocks.append(blk.reshape(1, -1))
    return np.concatenate(blocks, 0).astype(NPBF16)


# ---------------------------------------------------------------- builder

class _C:
    pass


def _emit_layer_mm(c, li, kind, lhsT, blob, ofs, out_cb):
    """Matmuls for one layer of a sweep, streaming weights in KSEG segments."""
    nc = c.nc
    K, N = _sweep_dims(kind, li)
    kt_n = _cdiv(K, 128)
    for grp in _groups(_chunks(N, CHUNK), NGROUP):
        accs = [c.apool.tile([128, CHUNK], F32, tag="acc", name="acc") for _ in grp]
        for seg0 in range(0, kt_n, KSEG):
            seg_n = min(KSEG, kt_n - seg0)
            nblk = seg_n * len(grp)
            wt = c.wpool.tile([128, KSEG * NGROUP * CHUNK], BF16, tag="wstream")
            nc.sync.dma_start(
                wt[:].rearrange("p (b f) -> p b f", f=CHUNK)[:, :nblk],
                blob[ofs[0]:ofs[0] + nblk].rearrange("b (p f) -> p b f", p=128))
            ofs[0] += nblk
            for si in range(seg_n):
                kt = seg0 + si
                kw = min(128, K - kt * 128)
                for gi, (n0, nw) in enumerate(grp):
                    bi = si * len(grp) + gi
                    nc.tensor.matmul(
                        accs[gi][:, :nw],
                        lhsT[:kw, kt * 128:kt * 128 + 128],
                        wt[:kw, bi * CHUNK:bi * CHUNK + nw],
                        start=(kt == 0), stop=(kt == kt_n - 1))
        for gi, (n0, nw) in enumerate(grp):
            out_cb(li, n0, nw, accs[gi])


def _emit_transpose(c, dst, src, width):
    """PE-transpose [128, width] bf16 src -> dst [128, kt_n*128] k-major tiles."""
    nc = c.nc
    kt_n = _cdiv(width, 128)
    for base in range(0, kt_n, 8):
        nt = min(8, kt_n - base)
        pt = c.tpool.tile([128, 8 * 128], BF16, tag="tr")
        kws = []
        for j in range(nt):
            kt = base + j
            kw = min(128, width - kt * 128)
            kws.append(kw)
            nc.tensor.transpose(
                pt[:kw, j * 128:j * 128 + 128],
                src[:, kt * 128:kt * 128 + kw],
                c.ident[:, :])
        if all(k == 128 for k in kws):
            nc.vector.tensor_copy(dst[:, base * 128:(base + nt) * 128],
                                  pt[:, :nt * 128])
        else:
            nfull = sum(1 for k in kws if k == 128)
            if nfull:
                nc.vector.tensor_copy(dst[:, base * 128:(base + nfull) * 128],
                                      pt[:, :nfull * 128])
            for j in range(nfull, nt):
                kw = kws[j]
                nc.vector.tensor_copy(
                    dst[:kw, (base + j) * 128:(base + j) * 128 + 128],
                    pt[:kw, j * 128:j * 128 + 128])


def build(steps):
    nc = bacc.Bacc("TRN2", target_bir_lowering=False, debug=False,
                   num_devices=N_CORES)
    c = _C()
    c.nc = nc

    obs_d = nc.dram_tensor("obs", [ROWS, SIZES[0]], F32, kind="ExternalInput").ap()
    obsT_d = nc.dram_tensor("obsT", [SIZES[0] // 128, 128 * ROWS], BF16,
                            kind="ExternalInput").ap()
    wt_d = nc.dram_tensor("wt_blob", [N_BLOCKS, 128 * CHUNK], BF16,
                          kind="ExternalInput").ap()
    wn_d = nc.dram_tensor("wn_blob", [N_BLOCKS, 128 * CHUNK], BF16,
                          kind="ExternalInput").ap()
    ident_d = nc.dram_tensor("ident", [128, 128], BF16, kind="ExternalInput").ap()
    noise_d = nc.dram_tensor("noise", [steps * ROWS, E_TOT], BF16,
                             kind="ExternalInput").ap()
    out_d = nc.dram_tensor("out", [ROWS, SIZES[4]], F32, kind="ExternalOutput").ap()

    with tile.TileContext(nc) as tc, contextlib.ExitStack() as st:
        c.wpool = st.enter_context(tc.tile_pool(name="wstream", bufs=3))
        c.apool = st.enter_context(tc.tile_pool(name="acc", bufs=6, space="PSUM"))
        c.tpool = st.enter_context(tc.tile_pool(name="tr", bufs=2, space="PSUM"))
        sp = st.enter_context(tc.tile_pool(name="state", bufs=1))
        sc = st.enter_context(tc.tile_pool(name="scratch", bufs=2))
        tp = st.enter_context(tc.tile_pool(name="tTpool", bufs=2))

        x = sp.tile([128, W_TOT], F32)
        e = sp.tile([128, E_TOT], BF16)
        ident = sp.tile([128, 128], BF16)
        c.ident = ident
        nc.sync.dma_start(ident[:], ident_d)
        eT = {lo: sp.tile([128, _cdiv(SIZES[lo], 128) * 128], BF16, tag=f"eT{lo}", name=f"eT{lo}")
              for lo in range(4)}

        def x_ap(li, n0=0, nw=None):
            nw = SIZES[li] if nw is None else nw
            o = int(XOFS[li]) + n0
            return x[:, o:o + nw]

        def e_ap(lo, n0=0, nw=None):
            nw = SIZES[lo] if nw is None else nw
            o = int(EOFS[lo]) + n0
            return e[:, o:o + nw]

        nc.sync.dma_start(x_ap(0), obs_d)

        # ---------------- init: x_li = x_{li-1} @ W_li
        def init_out(li, n0, nw, acc):
            nc.vector.tensor_copy(x_ap(li, n0, nw), acc[:, :nw])

        init_ofs = [0]
        xT_prev = sp.tile([128, 32 * 128], BF16, tag="xTinit", name="xTinit")
        nc.sync.dma_start(
            xT_prev[:, :SIZES[0]].rearrange("p (k f) -> p k f", f=128),
            obsT_d.rearrange("k (p f) -> p k f", p=128))
        for li in (1, 2, 3, 4):
            _emit_layer_mm(c, li, "init", xT_prev, wn_d, init_ofs, init_out)
            if li < 4:
                xT_prev = sp.tile([128, 32 * 128], BF16, tag="xTinit", name="xTinit")
                for (s0, swd) in _chunks(SIZES[li], 1024):
                    xb = sc.tile([128, 1024], BF16, tag="xbinit", name="xbinit")
                    nc.vector.tensor_copy(xb[:, :swd], x_ap(li, s0, swd))
                    _emit_transpose(c, xT_prev[:, s0:s0 + _cdiv(swd, 128) * 128], xb[:, :swd], swd)

        # ---------------- settling steps
        def step_body(i):
            fwd_ofs = [0]
            for li in (4, 3, 2, 1):
                tT = tp.tile([128, 32 * 128], BF16, tag="tT", name="tT")
                for (s0, swd) in _chunks(SIZES[li], 1024):
                    tb = sc.tile([128, 1024], BF16, tag="tcast", name="tcast")
                    for (n0, nw) in _chunks(swd, CHUNK):
                        nc.scalar.activation(tb[:, n0:n0 + nw],
                                             x_ap(li, s0 + n0, nw),
                                             mybir.ActivationFunctionType.Tanh)
                    _emit_transpose(c, tT[:, s0:s0 + _cdiv(swd, 128) * 128], tb[:, :swd], swd)

                def fwd_out(li_, n0, nw, acc, _lo=li - 1):
                    nz = sc.tile([128, CHUNK], BF16, tag="nz")
                    nc.sync.dma_start(
                        nz[:, :nw],
                        noise_d[bass.ts(i, ROWS),
                                int(EOFS[_lo]) + n0:int(EOFS[_lo]) + n0 + nw])
                    # e = (pred * -1 + x) + noise_eff
                    nc.vector.scalar_tensor_tensor(
                        e_ap(_lo, n0, nw), acc[:, :nw], -1.0, x_ap(_lo, n0, nw),
                        mybir.AluOpType.mult, mybir.AluOpType.add)
                    nc.vector.tensor_add(e_ap(_lo, n0, nw), e_ap(_lo, n0, nw),
                                         nz[:, :nw])

                _emit_layer_mm(c, li, "fwd", tT, wt_d, fwd_ofs, fwd_out)
                _emit_transpose(c, eT[li - 1], e_ap(li - 1), SIZES[li - 1])

            bwd_ofs = [0]

            def bwd_out(li, n0, nw, acc):
                t2 = sc.tile([128, CHUNK], F32, tag="t2")
                nc.scalar.activation(t2[:, :nw], x_ap(li, n0, nw),
                                     mybir.ActivationFunctionType.Tanh)
                nc.scalar.activation(t2[:, :nw], t2[:, :nw],
                                     mybir.ActivationFunctionType.Square)
                nc.vector.tensor_scalar(t2[:, :nw], t2[:, :nw], -1.0, 1.0,
                                        mybir.AluOpType.mult, mybir.AluOpType.add)
                gd = sc.tile([128, CHUNK], F32, tag="gd")
                nc.vector.tensor_mul(gd[:, :nw], acc[:, :nw], t2[:, :nw])
                if li < 4:  # e4 is identically zero in the reference
                    nc.vector.scalar_tensor_tensor(
                        gd[:, :nw], e_ap(li, n0, nw), -1.0, gd[:, :nw],
                        mybir.AluOpType.mult, mybir.AluOpType.add)
                nc.vector.scalar_tensor_tensor(
                    x_ap(li, n0, nw), gd[:, :nw], GAMMA, x_ap(li, n0, nw),
                    mybir.AluOpType.mult, mybir.AluOpType.add)

            for li in (1, 2, 3, 4):
                _emit_layer_mm(c, li, "bwd", eT[li - 1], wn_d, bwd_ofs, bwd_out)

        with tc.For_i(0, steps, 1, hint_engines=(mybir.EngineType.PE, mybir.EngineType.DVE, mybir.EngineType.Activation, mybir.EngineType.SP), staggered_reset=True) as i:
            step_body(i)

        nc.sync.dma_start(out_d, x_ap(4))
    nc.finalize()
    return nc


# ---------------------------------------------------------------- entry

_CACHE = {}


def kernel(**inputs):
    obs = np.asarray(inputs["obs"], np.float32)
    Ws = [np.asarray(inputs[f"W{i}"], np.float32) for i in range(1, 5)]
    bs = [np.asarray(inputs[f"b{i}"], np.float32) for i in range(1, 5)]
    steps = int(inputs["steps"])
    assert obs.shape == (BATCH, SIZES[0])

    if steps not in _CACHE:
        _CACHE[steps] = build(steps)
    nc = _CACHE[steps]

    noise = _noise_eff(steps, bs)  # [steps, 1024, E_TOT] bf16
    wt_blob = _pack_blob(Ws, "fwd")
    wn_blob = _pack_blob(Ws, "bwd")
    ident = np.eye(128, dtype=NPBF16)

    in_maps = []
    for cx in range(N_CORES):
        r0 = cx * ROWS
        obs_c = np.ascontiguousarray(obs[r0:r0 + ROWS])
        obsT_c = np.ascontiguousarray(
            obs_c.T.astype(NPBF16).reshape(SIZES[0] // 128, 128 * ROWS))
        nz_c = np.ascontiguousarray(
            noise[:, r0:r0 + ROWS, :]).reshape(steps * ROWS, E_TOT)
        in_maps.append({
            "obs": obs_c, "obsT": obsT_c, "wt_blob": wt_blob,
            "wn_blob": wn_blob, "ident": ident, "noise": nz_c,
        })

    res = bass_utils.run_bass_kernel_spmd(
        nc, in_maps, core_ids=list(range(N_CORES)), trace=False)
    return np.concatenate(
        [res.results[cx]["out"] for cx in range(N_CORES)], 0).astype(np.float32)
